# revision 38
# baseline (speedup 1.0000x reference)
"""Causal multi-head attention (B=2, S=2048, E=1024, H=16, D=64) on 8 trn2 NeuronCores.

Sharding: core c handles batch b = c // 4 and head group g = c % 4 (4 heads each).
Each core computes, for its batch and its 4 heads:
    q/k/v = x @ W[qkv][:, 256g:256g+256], causal attention, then the partial
    projection  out_heads @ Wp[256g:256g+256, :]  -> [2048, 1024].
Host gathers: out[b] = sum_g partial[b, g] + bp  (the "all-reduce" of the TP hint).

fp16 end-to-end (inputs cast on host; tolerance is 2e-2, fp16 lands ~7e-4):
  - x arrives in natural layout (4 big DMAs) and is PE-transposed (fp16 is
    1 cyc/row on the PE) into xT e-chunks; fp16 halves all DMA traffic.
  - kT/qT are [head-pair, S] tiles with the pair's heads stacked at
    partition 0/64; scores use tile_position=(64h, 0).
  - v is stored per (s-tile, head) as a [128, 128] slab [ones64 | v64]; the
    PV matmul then emits the softmax denominator on partitions 0..63 and the
    numerator on 64..127, so normalization is reciprocal+mul on the vector
    engine (no partition_broadcast, no copies).
  - attention is software-pipelined: scores for j-tile g+2 are emitted before
    PV(g); one filler job (qk chain / v tile / transpose pair / projection
    half) is popped per group so the scalar engine's exp stream stays hidden.
  - each score matmul's PSUM region starts 512-f32 aligned: two matmul groups
    packed into one PSUM bank at sub-bank offsets fail at runtime.
  - q-blocks run in order [0,384) [512,1024) [1024,1536) [1536,2048) [384,512):
    the last block is 128 q wide, so the end tail (norm+proj+DMA) is short.
"""

import os
import sys
import numpy as np

sys.path.insert(0, "/opt/trn_rl_repo")

import concourse.bass as bass
import concourse.bacc as bacc_mod
import concourse.mybir as mybir
import concourse.tile as tile
from concourse import library_config

F32 = mybir.dt.float32
F16 = mybir.dt.float16
P = 128

B = 2
S = 2048
E = 1024
NHEADS_TOTAL = 16
D = 64
N_CORES = 8
GROUPS = 4                        # head groups (tensor parallel)
HD = NHEADS_TOTAL * D // GROUPS   # 256 head-dims per core
NH = HD // D                      # heads per core (4)
NHP = HD // P                     # head pairs (2)
NST = S // P                      # s tiles (16)
NEC = E // P                      # e chunks (8)
NSC = S // 512                    # 512-wide s chunks (4)


def build_core_program(lower_isa=True):
    """One NeuronCore's program (SPMD: all 8 cores run this on different data)."""
    nc = bacc_mod.Bacc()
    x_d = nc.declare_dram_parameter("x", [S, E], F16, False)
    wq_d = nc.declare_dram_parameter("wq", [E, HD], F16, False)
    wk_d = nc.declare_dram_parameter("wk", [E, HD], F16, False)
    wv_d = nc.declare_dram_parameter("wv", [E, HD], F16, False)
    wp_d = nc.declare_dram_parameter("wp", [HD, E], F16, False)
    # identity comes in as data: building it with gpsimd memset+affine_select
    # would serialize the first PE transpose behind the ~10us gpsimd library
    # load DMA.
    id_d = nc.declare_dram_parameter("ident", [P, P], F16, False)
    y_d = nc.declare_dram_parameter("y", [S, E], F16, True)

    with tile.TileContext(nc) as tc:
        from contextlib import ExitStack
        with ExitStack() as ctx:
            persist = ctx.enter_context(tc.tile_pool(name="persist", bufs=1))

            ident = persist.tile([P, P], F16, tag="ident", name="ident")

            xT = [persist.tile([P, S], F16, tag=f"xT{ec}", name=f"xT{ec}")
                  for ec in range(NEC)]
            xn = [persist.tile([P, 4096], F16, tag=f"xn{sc}", name=f"xn{sc}")
                  for sc in range(NSC)]
            wsb = {nm: persist.tile([P, NEC * HD], F16, tag=nm, name=nm)
                   for nm in ("wq", "wk", "wv")}
            wp_sb = [persist.tile([P, E], F16, tag=f"wp{hp}", name=f"wp{hp}")
                     for hp in range(NHP)]
            qT = [persist.tile([P, S], F16, tag=f"qT{hp}", name=f"qT{hp}")
                  for hp in range(NHP)]
            kT = [persist.tile([P, S], F16, tag=f"kT{hp}", name=f"kT{hp}")
                  for hp in range(NHP)]
            # per (s-tile, head) slab [128, 128] = [ones 0:64 | v 64:128]
            v_ext = persist.tile([P, NST * NH * P], F16, tag="v_ext", name="v_ext")
            oT_all = [persist.tile([P, S], F16, tag=f"oT{hp}", name=f"oT{hp}")
                      for hp in range(NHP)]

            v_view = v_ext.rearrange("p (s h c) -> p s h c", s=NST, h=NH)
            nc.vector.memset(
                v_ext.rearrange("p (s c) -> p s c", s=NST * NH)[:, :, 0:D], 1.0)

            # ---------------- DMA issue ----------------
            # The x load is bandwidth-bound (~130 GB/s effective across the
            # rings), so what matters for the startup gap is CONTENTION for
            # the first bytes: s-chunk 0 is loaded in 4 e-pair slices (the
            # transpose-pair dependency granularity) so the first transposes
            # start after ~256KB instead of ~2MB, and chunks 2/3 are issued
            # later (after stage A0 emission).
            nc.sync.dma_start(out=ident[:], in_=id_d[:, :])
            def issue_xn(sc, ep=None):
                lo, hi = (0, 4096) if ep is None else (1024 * ep, 1024 * (ep + 1))
                nc.sync.dma_start(
                    out=xn[sc].rearrange("p (k e) -> p k e", k=4)
                        [:, :, lo // 4:hi // 4],
                    in_=x_d[512 * sc:512 * (sc + 1), HD * ep:HD * (ep + 1)]
                        .rearrange("(k p) e -> p k e", p=P) if ep is not None
                    else x_d[512 * sc:512 * (sc + 1), :]
                        .rearrange("(k p) e -> p k e", p=P),
                )
            def issue_w(nm, wd, hp):
                nc.scalar.dma_start(
                    out=wsb[nm].rearrange("p (c n) -> p c n", c=NEC)
                        [:, :, P * hp:P * (hp + 1)],
                    in_=wd[:, P * hp:P * (hp + 1)]
                        .rearrange("(c p) n -> p c n", p=P))
            for ep in range(4):
                issue_xn(0, ep)
            issue_w("wk", wk_d, 0)
            issue_w("wk", wk_d, 1)
            issue_xn(1, 0)
            issue_w("wq", wq_d, 0)
            issue_xn(1, 1)
            issue_w("wq", wq_d, 1)
            issue_xn(1, 2)
            nc.scalar.dma_start(
                out=wsb["wv"].rearrange("p (c n) -> p c n", c=NEC),
                in_=wv_d[:, :].rearrange("(c p) n -> p c n", p=P))
            issue_xn(1, 3)
            for hp in range(NHP):
                nc.scalar.dma_start(
                    out=wp_sb[hp], in_=wp_d[P * hp:P * (hp + 1), :])

            with tc.tile_pool(name="sT_ps", bufs=2, space="PSUM") as sT_ps, \
                 tc.tile_pool(name="oT_ps", bufs=1, space="PSUM") as oT_ps, \
                 tc.tile_pool(name="fill_ps", bufs=2, space="PSUM") as fill_ps, \
                 tc.tile_pool(name="pT", bufs=6) as pT_pool, \
                 tc.tile_pool(name="dr", bufs=6) as dr_pool, \
                 tc.tile_pool(name="ysb", bufs=3) as y_pool:

                # ---------- filler jobs (dependency-free PE work) ----------
                def tp_pair(sc, ep):
                    """transpose e-chunks 2ep, 2ep+1 of s-chunk sc into xT.
                    Shares the fill ring via bitcast (psum is bank-budgeted)."""
                    t = fill_ps.tile([P, 512], F32, tag="fill",
                                     name="fill").bitcast(F16)
                    for j in range(2):
                        ec = 2 * ep + j
                        for k in range(4):
                            nc.tensor.transpose(
                                t[:, 512 * j + P * k:512 * j + P * (k + 1)],
                                xn[sc][:, 1024 * k + P * ec:1024 * k + P * (ec + 1)],
                                ident[:])
                    for j in range(2):
                        nc.vector.tensor_copy(
                            xT[2 * ep + j][:, 512 * sc:512 * (sc + 1)],
                            t[:, 512 * j:512 * (j + 1)])

                def qk_chain(nm, hp, sc):
                    ps = fill_ps.tile([P, 512], F32, tag="fill", name="fill")
                    for ec in range(NEC):
                        nc.tensor.matmul(
                            ps[:],
                            wsb[nm][:, HD * ec + P * hp:HD * ec + P * (hp + 1)],
                            xT[ec][:, 512 * sc:512 * (sc + 1)],
                            start=(ec == 0), stop=(ec == NEC - 1),
                        )
                    dest = qT if nm == "wq" else kT
                    nc.vector.tensor_copy(
                        dest[hp][:, 512 * sc:512 * (sc + 1)], ps[:])

                def v_tile(st):
                    ps = fill_ps.tile([P, 512], F32, tag="fill", name="fill")
                    for ec in range(NEC):
                        nc.tensor.matmul(
                            ps[:, 0:HD],
                            xT[ec][:, P * st:P * (st + 1)],
                            wsb["wv"][:, HD * ec:HD * (ec + 1)],
                            start=(ec == 0), stop=(ec == NEC - 1),
                        )
                    nc.vector.tensor_copy(
                        v_view[:, st, :, D:P],
                        ps[:, 0:HD].rearrange("p (h c) -> p h c", h=NH),
                    )

                ysb_store = {}

                def proj_half(qt, nkk):
                    ps = fill_ps.tile([P, 512], F32, tag="fill", name="fill")
                    for hp in range(NHP):
                        nc.tensor.matmul(
                            ps[:],
                            oT_all[hp][:, P * qt:P * (qt + 1)],
                            wp_sb[hp][:, 512 * nkk:512 * (nkk + 1)],
                            start=(hp == 0), stop=(hp == NHP - 1),
                        )
                    ysb = ysb_store[qt]
                    nc.vector.tensor_copy(ysb[:, 512 * nkk:512 * (nkk + 1)], ps[:])
                    if nkk == 1:
                        nc.sync.dma_start(out=y_d[P * qt:P * (qt + 1), :], in_=ysb)

                def proj_jobs(qts):
                    jobs = []
                    for qt in qts:
                        ysb_store[qt] = y_pool.tile([P, E], F16, tag="ysb",
                                                    name=f"ysb{qt}")
                        jobs.append(lambda qt=qt: proj_half(qt, 0))
                        jobs.append(lambda qt=qt: proj_half(qt, 1))
                    return jobs

                # ---------- attention block ----------
                def attn_block(q0, qw, fillers, pop_n=1):
                    """fillers: thunks popped pop_n per j-tile group."""
                    n_j = (q0 + qw) // P
                    fill_i = [0]

                    def pop_filler():
                        for _ in range(pop_n):
                            if fill_i[0] < len(fillers):
                                fillers[fill_i[0]]()
                                fill_i[0] += 1

                    for hp in range(NHP):
                        sT = {}
                        pT = {}

                        def emit_S(js):
                            cm = max(0, P * js - q0)
                            t = sT_ps.tile([P, 1024], F32, tag="sT", name="sT")
                            sT[js] = (t, cm)
                            for h in range(2):
                                lo = D * h
                                nc.tensor.matmul(
                                    t[:, 512 * h + cm:512 * h + qw],
                                    kT[hp][lo:lo + D, P * js:P * (js + 1)],
                                    qT[hp][lo:lo + D, q0 + cm:q0 + qw],
                                    start=True, stop=True,
                                    tile_position=(lo, 0),
                                )

                        def emit_exp_mask(js):
                            # pT mirrors the psum layout (head h at 512h), so
                            # one exp spans both heads; the dead middle
                            # [qw, 512+cm) holds exp(garbage) and is never
                            # read.  One affine_select masks both heads via a
                            # zero-step h dimension.
                            t, cm = sT[js]
                            p = pT_pool.tile([P, 1024], F16, tag="pT", name="pT")
                            pT[js] = (p, cm)
                            if qw <= 256:
                                for h in range(2):
                                    nc.scalar.activation(
                                        p[:, 512 * h + cm:512 * h + qw],
                                        t[:, 512 * h + cm:512 * h + qw],
                                        mybir.ActivationFunctionType.Exp,
                                        scale=0.125)
                            else:
                                nc.scalar.activation(
                                    p[:, cm:512 + qw], t[:, cm:512 + qw],
                                    mybir.ActivationFunctionType.Exp, scale=0.125)
                            ce = min(cm + P, qw)
                            if P * js + P > q0:  # diagonal tile: causal mask
                                w = ce - cm
                                pv = p.rearrange("p (h c) -> p h c", h=2)
                                nc.gpsimd.affine_select(
                                    out=pv[:, :, cm:ce],
                                    in_=pv[:, :, cm:ce],
                                    pattern=[[0, 2], [1, w]],
                                    compare_op=mybir.AluOpType.is_ge,
                                    fill=0.0,
                                    base=q0 + cm - P * js,
                                    channel_multiplier=-1,
                                )

                        def emit_PV(js, oT2):
                            p, cm = pT.pop(js)
                            sT.pop(js)
                            for h in range(2):
                                hl = 2 * hp + h
                                nc.tensor.matmul(
                                    oT2[:, 512 * h + cm:512 * h + qw],
                                    v_view[:, js, hl, :],
                                    p[:, 512 * h + cm:512 * h + qw],
                                    start=(js == 0), stop=(js == n_j - 1),
                                )

                        oT2 = oT_ps.tile([P, 1024], F32, tag="oT", name="oT")
                        emit_S(0)
                        if n_j > 1:
                            emit_S(1)
                        for js in range(n_j):
                            emit_exp_mask(js)
                            if js + 2 < n_j:
                                emit_S(js + 2)
                            emit_PV(js, oT2)
                            pop_filler()
                        # normalize: oT2 rows 0:64 = denominator (ones cols),
                        # rows 64:128 = numerator, per 512-half per head.
                        for h in range(2):
                            dr = dr_pool.tile([D, 512], F32, tag="dr", name="dr")
                            nc.vector.reciprocal_approx_fast(
                                dr[:, 0:qw], oT2[0:D, 512 * h:512 * h + qw])
                            nc.vector.tensor_mul(
                                oT_all[hp][D * h:D * (h + 1), q0:q0 + qw],
                                oT2[D:P, 512 * h:512 * h + qw], dr[:, 0:qw])
                    while fill_i[0] < len(fillers):
                        fillers[fill_i[0]]()
                        fill_i[0] += 1

                def TP(sc, ep):
                    return lambda: tp_pair(sc, ep)

                def QK(nm, hp, sc):
                    return lambda: qk_chain(nm, hp, sc)

                def V(st):
                    return lambda: v_tile(st)

                # ---------- schedule ----------
                # pre-0a: transposes sc0, qk chains sc0 (with sc1 transposes
                # interleaved to hide the fill-copy latency), v0..v2
                for ep in range(4):
                    tp_pair(0, ep)
                qk_chain("wk", 0, 0); tp_pair(1, 0)
                qk_chain("wk", 1, 0); tp_pair(1, 1)
                qk_chain("wq", 0, 0); tp_pair(1, 2)
                qk_chain("wq", 1, 0); tp_pair(1, 3)
                v_tile(0)
                v_tile(1)
                v_tile(2)

                # block 0a (q 0..384, 3x2 groups): qk(sc1) fillers
                f0a = [QK("wk", 0, 1), QK("wk", 1, 1), QK("wq", 0, 1),
                       QK("wq", 1, 1), V(3)]
                attn_block(0, 384, f0a)

                # block 1 (q 512..1024, 16 slots): xn2 issue + v4..7 first
                # (own j-tiles), then transposes sc2, qk(sc2), v8,9.  The
                # deferred x-chunk issues ride the filler stream so their
                # descriptor generation hits the sync queue when it has slack.
                f1 = [lambda: issue_xn(2), V(4), V(5), V(6), V(7),
                      TP(2, 0), TP(2, 1), TP(2, 2), TP(2, 3),
                      QK("wk", 0, 2), QK("wk", 1, 2), QK("wq", 0, 2),
                      QK("wq", 1, 2), V(8), V(9)]
                attn_block(512, 512, f1)

                # block 2 (q 1024..1536, 24 slots): v10,11, transposes sc3,
                # qk(sc3), proj(qt0,1)
                pj01 = proj_jobs([0, 1])
                f2 = [lambda: issue_xn(3), V(10), V(11)] + pj01[0:2] + [
                      TP(3, 0), TP(3, 1), TP(3, 2), TP(3, 3)] + pj01[2:4] + [
                      QK("wk", 0, 3), QK("wk", 1, 3), QK("wq", 0, 3),
                      QK("wq", 1, 3)]
                attn_block(1024, 512, f2)

                # block 3 (q 1536..2048, 32 slots): v12..15 early, then
                # projections for ready columns (qt2 from 0a, 4..7)
                f3 = [V(12), V(13), V(14), V(15)]
                f3 += proj_jobs([2, 4, 5, 6, 7])
                attn_block(1536, 512, f3)

                # block 0b (q 384..512, 8 slots): proj(qt8..15), 2 per slot
                f0b = proj_jobs([8, 9, 10, 11, 12, 13, 14, 15])
                attn_block(384, 128, f0b, pop_n=2)

                # tail: qt3 only — both halves in one sT tile (attention is
                # done, the pool is free), copies split scalar/vector
                for qt in (3,):
                    ysb = y_pool.tile([P, E], F16, tag="ysb", name=f"ysb{qt}")
                    t = sT_ps.tile([P, 1024], F32, tag="sT", name="sT")
                    for nkk in range(2):
                        for hp in range(NHP):
                            nc.tensor.matmul(
                                t[:, 512 * nkk:512 * (nkk + 1)],
                                oT_all[hp][:, P * qt:P * (qt + 1)],
                                wp_sb[hp][:, 512 * nkk:512 * (nkk + 1)],
                                start=(hp == 0), stop=(hp == NHP - 1),
                            )
                    for nkk in range(2):
                        src = t[:, 512 * nkk:512 * (nkk + 1)]
                        dst = ysb[:, 512 * nkk:512 * (nkk + 1)]
                        if nkk == 0:
                            nc.scalar.copy(dst, src)
                        else:
                            nc.vector.tensor_copy(dst, src)
                        eng = nc.sync if nkk == 0 else nc.scalar
                        eng.dma_start(
                            out=y_d[P * qt:P * (qt + 1), 512 * nkk:512 * (nkk + 1)],
                            in_=dst)

    if lower_isa:
        nc.finalize()
    return nc


_CACHED_NC = None


def _get_nc():
    global _CACHED_NC
    if _CACHED_NC is None:
        _CACHED_NC = build_core_program()
    return _CACHED_NC


def shard_inputs(x, Wq, Wk, Wv, Wp):
    in_maps = []
    x16 = [np.ascontiguousarray(x[b], dtype=np.float16) for b in range(B)]
    wq16 = np.asarray(Wq, dtype=np.float16)
    wk16 = np.asarray(Wk, dtype=np.float16)
    wv16 = np.asarray(Wv, dtype=np.float16)
    wp16 = np.asarray(Wp, dtype=np.float16)
    ident = np.eye(P, dtype=np.float16)
    for core in range(N_CORES):
        b, g = core // GROUPS, core % GROUPS
        sl = slice(HD * g, HD * (g + 1))
        in_maps.append({
            "x": x16[b],
            "wq": np.ascontiguousarray(wq16[:, sl]),
            "wk": np.ascontiguousarray(wk16[:, sl]),
            "wv": np.ascontiguousarray(wv16[:, sl]),
            "wp": np.ascontiguousarray(wp16[sl, :]),
            "ident": ident,
        })
    return in_maps


def _ensure_ntff_hook():
    """Provide antenv.axon_hooks (missing in this image) so trace=True can
    collect NTFF profiles through libaxon_pjrt's nrt-profile C ABI."""
    import types
    try:
        from antenv.axon_hooks import get_axon_ntff_profile_hook  # noqa: F401
        return
    except ImportError:
        pass
    import antenv
    mod = types.ModuleType("antenv.axon_hooks")
    mod._hook = None
    def set_axon_ntff_profile_hook(h):
        mod._hook = h
    def get_axon_ntff_profile_hook():
        return mod._hook
    mod.set_axon_ntff_profile_hook = set_axon_ntff_profile_hook
    mod.get_axon_ntff_profile_hook = get_axon_ntff_profile_hook
    sys.modules["antenv.axon_hooks"] = mod
    antenv.axon_hooks = mod
    try:
        from trn_agent_boot.trn_boot import _ntff_profile_via_ctypes
        mod._hook = _ntff_profile_via_ctypes("/opt/axon/libaxon_pjrt.so")
    except Exception as e:  # degrade: tracing skipped, run still works
        print(f"ntff hook setup failed: {e}", file=sys.stderr)


def run(inputs, trace=False, **spmd_kwargs):
    """Returns (full_output [B,S,E], BassKernelResults)."""
    from concourse.bass_utils import run_bass_kernel_spmd
    if trace:
        _ensure_ntff_hook()
    x = np.asarray(inputs["x"], dtype=np.float32)
    Wq = np.asarray(inputs["Wq"], dtype=np.float32)
    Wk = np.asarray(inputs["Wk"], dtype=np.float32)
    Wv = np.asarray(inputs["Wv"], dtype=np.float32)
    Wp = np.asarray(inputs["Wp"], dtype=np.float32)
    bp = np.asarray(inputs["bp"], dtype=np.float32)

    nc = _get_nc()
    in_maps = shard_inputs(x, Wq, Wk, Wv, Wp)
    res = run_bass_kernel_spmd(nc, in_maps, list(range(N_CORES)),
                               trace=trace, **spmd_kwargs)
    out = np.zeros((B, S, E), dtype=np.float32)
    for core in range(N_CORES):
        out[core // GROUPS] += res.results[core]["y"].astype(np.float32)
    out += bp[None, None, :]
    return out, res


def kernel(x, Wq, Wk, Wv, Wp, bp):
    out, _ = run({"x": x, "Wq": Wq, "Wk": Wk, "Wv": Wv, "Wp": Wp, "bp": bp})
    return out
